# revision 22
# baseline (speedup 1.0000x reference)
"""Trainium2 Bass kernel for nn_ExpertClassifierBank.

Computes, for pooled [B,K,D], expert weights [E,C,D], indices [K], log_scales [E]:
    x = l2norm(pooled, axis=-1)
    w = l2norm(weights[idx], axis=-1)
    out[b,k,c] = min(exp(log_scales[idx[k]]), 100) * dot(x[b,k], w[k,c])

Sharding: data-parallel over batch B across 8 NeuronCores (512 rows each);
the gathered expert weight bank is replicated.

Host folds the per-expert normalizer min(exp(ls),100)/||w_kc|| into the
weight bank (pure weight preprocessing), so the device computes
    lg[c,b]  = sum_d w_eff[k,c,d] * x[b,k,d]      (bf16 matmuls)
    ss[k,b]  = sum_d x[b,k,d]^2                   (fp8 DoubleRow matmuls)
    out[c,b] = lg[c,b] / sqrt(ss[k,b])            (recip+sqrt, f32r selector
                                                   broadcast matmul, DVE mult)

Key scheduling facts this build is shaped around (measured):
  - 2 HWDGE queues (sync+scalar) sustain ~400 B/ns aggregate; one alone ~212.
  - the allocator hands out 8 DMA-completion semaphores round-robin in
    scheduled order; alternating emission between the queues keeps every
    recycled-sem wait on an already-finished same-queue DMA.
  - DMA triggers cost ~600ns of issuing-engine time each -> w rides in
    k-pair packs; outputs go mostly to the (idle) sync engine's queue.
  - squares at ~1 elem/cycle/partition on ACT/DVE (fp8 out), ~1.9x that on
    gpsimd; DoubleRow fp8 matmuls keep the PE reduce at 0.5 cyc/row.
  - one ACT function table covers sqrt+square+copy; a dummy Sqrt first
    forces it to load once, early, off the critical path.
  - x0/x1 and x6/x7 land as half-tiles so the PE starts ~11us and the
    k7 tail chain starts right when the stream ends; k7's last square
    pair runs on DVE (not the slower gpsimd).
"""

import time

import numpy as np
import ml_dtypes

import concourse.bass as bass
import concourse.mybir as mybir
import concourse.tile as tile
from concourse import bacc
from concourse.bass_utils import run_bass_kernel_spmd

N_CORES = 8
B, K, D, C, E = 4096, 8, 1024, 100, 16
BLOC = B // N_CORES  # 512
P = 128
DC = D // P  # 8 d-chunks
HALF = 4  # k-batch size for the f pipeline
JA = 4  # first-half d-chunks (split DMA boundary)
JP = DC // 2  # fp8 row-pairs per k

F32 = mybir.dt.float32
F32R = mybir.dt.float32r
BF16 = mybir.dt.bfloat16
F8 = mybir.dt.float8e4
AF = mybir.ActivationFunctionType
MULT = mybir.AluOpType.mult
DROW = mybir.MatmulPerfMode.DoubleRow
NPBF16 = ml_dtypes.bfloat16
NPF8 = ml_dtypes.float8_e4m3

_CACHE = {}

LAST_RESULT = None
LAST_WALL_NS = None


def _build():
    nc = bacc.Bacc(
        "TRN2", target_bir_lowering=False, debug=False, num_devices=N_CORES
    )

    xt = nc.dram_tensor("xt", [K, P, DC, BLOC], BF16, kind="ExternalInput").ap()
    # w packed in k-pairs: one DMA covers two adjacent k's
    wt = nc.dram_tensor("wt", [K // 2, P, 2, DC, C], BF16,
                        kind="ExternalInput").ap()
    sel4 = nc.dram_tensor("sel4", [P, 2, HALF, HALF], F8,
                          kind="ExternalInput").ap()
    selc4 = nc.dram_tensor("selc4", [HALF, HALF, C], F32R,
                           kind="ExternalInput").ap()
    out = nc.dram_tensor("out", [K, C, BLOC], BF16, kind="ExternalOutput").ap()

    with tile.TileContext(nc) as tc:
        with (
            tc.tile_pool(name="const", bufs=1) as cpool,
            tc.tile_pool(name="xres", bufs=K) as xpool,
            tc.tile_pool(name="wres", bufs=K // 2) as wpool,
            tc.tile_pool(name="x2", bufs=K) as x2pool,
            tc.tile_pool(name="lgs", bufs=K) as lgspool,
            tc.tile_pool(name="osb", bufs=K) as opool,
            tc.tile_pool(name="fx", bufs=6) as fpool,
        ):
            with tc.high_priority():
                sel4_sb = cpool.tile([P, 2, HALF, HALF], F8)
                nc.gpsimd.dma_start(sel4_sb[:], sel4[:])
                selc4_sb = cpool.tile([HALF, HALF, C], F32R)
                nc.gpsimd.dma_start(selc4_sb[:], selc4[:])

                wp_sbs = [None] * (K // 2)
                x_sbs = [None] * K
                for kp in range(K // 2):
                    wp_sbs[kp] = wpool.tile([P, 2, DC, C], BF16, tag="w",
                                            name=f"wp{kp}")
                for k in range(K):
                    x_sbs[k] = xpool.tile([P, DC, BLOC], BF16, tag="x",
                                          name=f"x{k}")

                def eng_of(k):
                    return nc.sync if k % 2 == 0 else nc.scalar

                # queue content order (each queue):
                #   x_first_a, wpair, x_first_b, x, wpair, x, x_last_a/b
                # emission alternates queues so recycled DMA sems always
                # point at an earlier same-queue (or long-done) DMA.
                for k in (0, 1):
                    eng_of(k).dma_start(x_sbs[k][:, :JA], xt[k][:, :JA])
                eng_of(0).dma_start(wp_sbs[0][:], wt[0])
                eng_of(1).dma_start(wp_sbs[1][:], wt[1])
                for k in (0, 1):
                    eng_of(k).dma_start(x_sbs[k][:, JA:], xt[k][:, JA:])
                for k in (2, 3):
                    eng_of(k).dma_start(x_sbs[k][:], xt[k])
                eng_of(0).dma_start(wp_sbs[2][:], wt[2])
                eng_of(1).dma_start(wp_sbs[3][:], wt[3])
                for k in (4, 5):
                    eng_of(k).dma_start(x_sbs[k][:], xt[k])
                for k in (6, 7):
                    eng_of(k).dma_start(x_sbs[k][:, :JA], xt[k][:, :JA])
                for k in (6, 7):
                    eng_of(k).dma_start(x_sbs[k][:, JA:], xt[k][:, JA:])

            def w_ap(k):
                return wp_sbs[k // 2][:, k % 2]

            # dummy Sqrt: loads the sqrt+square+copy ACT table once, early
            warm = cpool.tile([1, HALF], F32)
            nc.scalar.activation(
                warm[:], selc4_sb[0:1, 0, :HALF].bitcast(F32), AF.Sqrt
            )

            psum_ctx = (
                tc.tile_pool(name="pss", bufs=2, space="PSUM"),
                tc.tile_pool(name="plog", bufs=3, space="PSUM"),
                tc.tile_pool(name="pf", bufs=2, space="PSUM"),
                tc.tile_pool(name="pwarm", bufs=1, space="PSUM"),
            )
            pss = psum_ctx[0].__enter__()
            plog = psum_ctx[1].__enter__()
            pf = psum_ctx[2].__enter__()
            pwarm = psum_ctx[3].__enter__()
            # scratch bank for PE clock-warmer matmuls: the tensor engine
            # ramps 1.2->2.4GHz only after ~3us of continuous execution,
            # and DMA-paced arrival gaps keep resetting it. Warmers rerun
            # already-resident data into a never-read bank so the PE
            # stays busy (and fast) across the gaps.
            warm_ps = pwarm.tile([C, BLOC], F32, tag="warm")

            sss = []
            fx_sbs = []
            lgs_sbs = {}

            def emit_fb_out(kk):
                half = kk // HALF
                ii = kk % HALF
                fb = pf.tile([C, BLOC], F32, tag="fb", name=f"fb{kk}")
                nc.tensor.matmul(
                    fb[:],
                    lhsT=selc4_sb[:, ii, :],
                    rhs=fx_sbs[half][:],
                    start=True, stop=True,
                    skip_group_check=True,
                )
                o_sb = opool.tile([C, BLOC], BF16, tag="o", name=f"o{kk}")
                nc.vector.tensor_tensor(o_sb[:], lgs_sbs[kk][:], fb[:], MULT)
                eng = nc.sync if kk < 6 else nc.scalar
                eng.dma_start(out[kk], o_sb[:])

            def emit_fchain(half):
                ss = sss[half]
                recx = fpool.tile([HALF, BLOC], F32, tag="recx",
                                  name=f"recx{half}")
                scr = fpool.tile([HALF, BLOC], F32, tag="rscr",
                                 name=f"rscr{half}")
                nc.vector.reciprocal_approx_accurate(recx[:], ss[:], scr[:])
                fx = fpool.tile([HALF, BLOC], F32R, tag="fx", name=f"fx{half}")
                nc.scalar.activation(fx[:], recx[:], AF.Sqrt)
                fx_sbs.append(fx)

            def ss_mm(ss, x2, i, jp):
                nc.tensor.matmul(
                    ss[:],
                    lhsT=sel4_sb[:, :, i, :],
                    rhs=x2[:, jp],
                    start=(i == 0 and jp == 0),
                    stop=(i == HALF - 1 and jp == JP - 1),
                    perf_mode=DROW,
                    skip_group_check=True,
                )

            for k in range(K):
                half, i = divmod(k, HALF)
                if i == 0:
                    ss = pss.tile([HALF, BLOC], F32, tag="ss", name=f"ss{half}")
                    sss.append(ss)
                ss = sss[half]
                if k == HALF:
                    # h0 f-chain traced here: recip gates on k3's last ss
                    emit_fchain(0)
                # squares into fp8 row-pairs: ACT jp0, DVE jp1 (both from
                # the first-arriving x half), DVE jp2, gpsimd jp3 (DVE for
                # k7: gpsimd is slower and jp3(k7) gates the tail chain)
                x2 = x2pool.tile([P, JP, 2, BLOC], F8, tag="x2", name=f"x2_{k}")
                nc.scalar.activation(x2[:, 0:1], x_sbs[k][:, 0:2], AF.Square)
                nc.vector.tensor_tensor(
                    x2[:, 1:2], x_sbs[k][:, 2:4], x_sbs[k][:, 2:4], MULT
                )
                nc.vector.tensor_tensor(
                    x2[:, 2:3], x_sbs[k][:, 4:6], x_sbs[k][:, 4:6], MULT
                )
                if k < K - 1:
                    nc.gpsimd.tensor_tensor(
                        x2[:, 3:4], x_sbs[k][:, 6:8], x_sbs[k][:, 6:8], MULT
                    )
                else:
                    nc.vector.tensor_tensor(
                        x2[:, 3:4], x_sbs[k][:, 6:8], x_sbs[k][:, 6:8], MULT
                    )
                # first-half ss + main matmuls can run before the second
                # x half lands
                ss_mm(ss, x2, i, 0)
                ss_mm(ss, x2, i, 1)
                lg = plog.tile([C, BLOC], F32, tag="lg", name=f"lg{k}")
                for j in range(JA):
                    nc.tensor.matmul(
                        lg[:],
                        lhsT=w_ap(k)[:, j, :],
                        rhs=x_sbs[k][:, j],
                        start=(j == 0),
                        stop=False,
                        skip_group_check=True,
                    )
                ss_mm(ss, x2, i, 2)
                ss_mm(ss, x2, i, 3)
                if k == K - 1:
                    # h1 f-chain gates only on k7's ss, traced before the
                    # remaining main matmuls so it overlaps them
                    emit_fchain(1)
                for j in range(JA, DC):
                    nc.tensor.matmul(
                        lg[:],
                        lhsT=w_ap(k)[:, j, :],
                        rhs=x_sbs[k][:, j],
                        start=False,
                        stop=(j == DC - 1),
                        skip_group_check=True,
                    )
                lgs = lgspool.tile([C, BLOC], F32, tag="lgs", name=f"lgs{k}")
                nc.scalar.activation(lgs[:], lg[:], AF.Copy)
                lgs_sbs[k] = lgs
                if half == 1:
                    # deferred half0 outputs: one per k=4..7
                    emit_fb_out(k - HALF)
                if k < K - 2:
                    # clock warmers: re-stream this k's already-resident
                    # data while waiting for the next tiles to land
                    for j in range(4):
                        nc.tensor.matmul(
                            warm_ps[:],
                            lhsT=w_ap(k)[:, j, :],
                            rhs=x_sbs[k][:, j],
                            start=True, stop=True,
                            skip_group_check=True,
                        )
            for kk in range(HALF, K):
                emit_fb_out(kk)

            for c in reversed(psum_ctx):
                c.__exit__(None, None, None)

    nc.compile()
    return nc


def _host_prep(pooled, active_expert_indices, weights, log_scales):
    idx = np.asarray(active_expert_indices).astype(np.int64)
    pooled = np.asarray(pooled, dtype=np.float32)
    weights = np.asarray(weights, dtype=np.float32)
    log_scales = np.asarray(log_scales, dtype=np.float32)

    # x: [B,K,D] -> bf16 -> per-core [K, P, DC, BLOC]  (k, d, j, b)
    pb = pooled.astype(NPBF16)
    xt_all = np.ascontiguousarray(
        pb.reshape(N_CORES, BLOC, K, DC, P).transpose(0, 2, 4, 3, 1)
    )
    # w_eff: gather + fold cosine normalizer and clamped logit scale
    wg = weights[idx]  # [K, C, D]
    nrm = np.sqrt(np.sum(wg * wg, axis=-1, keepdims=True))
    scale = np.minimum(np.exp(log_scales[idx]), 100.0)[:, None, None]
    weff = (wg / np.maximum(nrm, 1e-12) * scale).astype(NPBF16)
    # [K,C,D] -> [K/2, P, 2, DC, C] k-pair packs
    wt = np.ascontiguousarray(
        weff.reshape(K // 2, 2, C, DC, P).transpose(0, 4, 1, 3, 2)
    )

    sel4 = np.zeros((P, 2, HALF, HALF), NPF8)
    for i in range(HALF):
        sel4[:, :, i, i] = 1.0
    selc4 = np.zeros((HALF, HALF, C), np.float32)
    for i in range(HALF):
        selc4[i, i, :] = 1.0

    shared = {"wt": wt, "sel4": sel4, "selc4": selc4}
    return [dict(shared, xt=np.ascontiguousarray(xt_all[co]))
            for co in range(N_CORES)]


def kernel(pooled, active_expert_indices, weights, log_scales):
    global LAST_RESULT, LAST_WALL_NS
    if "nc" not in _CACHE:
        _CACHE["nc"] = _build()
    nc = _CACHE["nc"]

    in_maps = _host_prep(pooled, active_expert_indices, weights, log_scales)

    t0 = time.perf_counter_ns()
    res = run_bass_kernel_spmd(nc, in_maps, core_ids=list(range(N_CORES)))
    LAST_WALL_NS = time.perf_counter_ns() - t0
    LAST_RESULT = res

    full = np.stack(
        [res.results[co]["out"].astype(np.float32) for co in range(N_CORES)]
    )
    return np.ascontiguousarray(
        full.transpose(0, 3, 1, 2).reshape(B, K, C)
    )


# revision 24
# speedup vs baseline: 1.0690x; 1.0690x over previous
"""Trainium2 Bass kernel for nn_ExpertClassifierBank.

Computes, for pooled [B,K,D], expert weights [E,C,D], indices [K], log_scales [E]:
    x = l2norm(pooled, axis=-1)
    w = l2norm(weights[idx], axis=-1)
    out[b,k,c] = min(exp(log_scales[idx[k]]), 100) * dot(x[b,k], w[k,c])

Sharding: data-parallel over batch B across 8 NeuronCores (512 rows each);
the gathered expert weight bank is replicated.

Host folds the per-expert normalizer min(exp(ls),100)/||w_kc|| into the
weight bank (pure weight preprocessing), so the device computes
    lg[c,b]  = sum_d w_eff[k,c,d] * x[b,k,d]      (bf16 matmuls)
    ss[k,b]  = sum_d x[b,k,d]^2                   (fp8 DoubleRow matmuls)
    out[c,b] = lg[c,b] / sqrt(ss[k,b])            (recip+sqrt, f32r selector
                                                   broadcast matmul, DVE mult)

Key scheduling facts this build is shaped around (measured):
  - 2 HWDGE queues (sync+scalar) sustain ~400 B/ns aggregate; one alone ~212.
  - the allocator hands out 8 DMA-completion semaphores round-robin in
    scheduled order; alternating emission between the queues keeps every
    recycled-sem wait on an already-finished same-queue DMA.
  - DMA triggers cost ~600ns of issuing-engine time each -> w rides in
    k-pair packs; outputs go mostly to the (idle) sync engine's queue.
  - squares at ~1 elem/cycle/partition on ACT/DVE (fp8 out), ~1.9x that on
    gpsimd; DoubleRow fp8 matmuls keep the PE reduce at 0.5 cyc/row.
  - one ACT function table covers sqrt+square+copy; a dummy Sqrt first
    forces it to load once, early, off the critical path.
  - x0/x1 and x6/x7 land as half-tiles so the PE starts ~11us and the
    k7 tail chain starts right when the stream ends; k7's last square
    pair runs on DVE (not the slower gpsimd).
"""

import time

import numpy as np
import ml_dtypes

import concourse.bass as bass
import concourse.mybir as mybir
import concourse.tile as tile
from concourse import bacc
from concourse.bass_utils import run_bass_kernel_spmd

N_CORES = 8
B, K, D, C, E = 4096, 8, 1024, 100, 16
BLOC = B // N_CORES  # 512
P = 128
DC = D // P  # 8 d-chunks
HALF = 4  # k-batch size for the f pipeline
JA = 4  # first-half d-chunks (split DMA boundary)
JP = DC // 2  # fp8 row-pairs per k

F32 = mybir.dt.float32
F32R = mybir.dt.float32r
BF16 = mybir.dt.bfloat16
F8 = mybir.dt.float8e4
AF = mybir.ActivationFunctionType
MULT = mybir.AluOpType.mult
DROW = mybir.MatmulPerfMode.DoubleRow
NPBF16 = ml_dtypes.bfloat16
NPF8 = ml_dtypes.float8_e4m3

_CACHE = {}

LAST_RESULT = None
LAST_WALL_NS = None


def _build():
    nc = bacc.Bacc(
        "TRN2", target_bir_lowering=False, debug=False, num_devices=N_CORES
    )

    xt = nc.dram_tensor("xt", [K, P, DC, BLOC], BF16, kind="ExternalInput").ap()
    # w packed in k-pairs: one DMA covers two adjacent k's
    wt = nc.dram_tensor("wt", [K // 2, P, 2, DC, C], BF16,
                        kind="ExternalInput").ap()
    sel4 = nc.dram_tensor("sel4", [P, 2, HALF, HALF], F8,
                          kind="ExternalInput").ap()
    selc4 = nc.dram_tensor("selc4", [HALF, HALF, C], F32R,
                           kind="ExternalInput").ap()
    out = nc.dram_tensor("out", [K, C, BLOC], BF16, kind="ExternalOutput").ap()

    with tile.TileContext(nc) as tc:
        with (
            tc.tile_pool(name="const", bufs=1) as cpool,
            tc.tile_pool(name="xres", bufs=K) as xpool,
            tc.tile_pool(name="wres", bufs=K // 2) as wpool,
            tc.tile_pool(name="x2", bufs=K) as x2pool,
            tc.tile_pool(name="lgs", bufs=K) as lgspool,
            tc.tile_pool(name="osb", bufs=K) as opool,
            tc.tile_pool(name="fx", bufs=6) as fpool,
        ):
            with tc.high_priority():
                sel4_sb = cpool.tile([P, 2, HALF, HALF], F8)
                nc.gpsimd.dma_start(sel4_sb[:], sel4[:])
                selc4_sb = cpool.tile([HALF, HALF, C], F32R)
                nc.gpsimd.dma_start(selc4_sb[:], selc4[:])

                wp_sbs = [None] * (K // 2)
                x_sbs = [None] * K
                for kp in range(K // 2):
                    wp_sbs[kp] = wpool.tile([P, 2, DC, C], BF16, tag="w",
                                            name=f"wp{kp}")
                for k in range(K):
                    x_sbs[k] = xpool.tile([P, DC, BLOC], BF16, tag="x",
                                          name=f"x{k}")

                def eng_of(k):
                    return nc.sync if k % 2 == 0 else nc.scalar

                # queue content order (each queue):
                #   x_first_a, wpair, x_first_b, x, wpair, x, x_last_a/b
                # emission alternates queues so recycled DMA sems always
                # point at an earlier same-queue (or long-done) DMA.
                for k in (0, 1):
                    eng_of(k).dma_start(x_sbs[k][:, :JA], xt[k][:, :JA])
                eng_of(0).dma_start(wp_sbs[0][:], wt[0])
                eng_of(1).dma_start(wp_sbs[1][:], wt[1])
                for k in (0, 1):
                    eng_of(k).dma_start(x_sbs[k][:, JA:], xt[k][:, JA:])
                for k in (2, 3):
                    eng_of(k).dma_start(x_sbs[k][:], xt[k])
                eng_of(0).dma_start(wp_sbs[2][:], wt[2])
                eng_of(1).dma_start(wp_sbs[3][:], wt[3])
                for k in (4, 5):
                    eng_of(k).dma_start(x_sbs[k][:], xt[k])
                for k in (6, 7):
                    eng_of(k).dma_start(x_sbs[k][:, :JA], xt[k][:, :JA])
                for k in (6, 7):
                    eng_of(k).dma_start(x_sbs[k][:, JA:], xt[k][:, JA:])

            def w_ap(k):
                return wp_sbs[k // 2][:, k % 2]

            # dummy Sqrt: loads the sqrt+square+copy ACT table once, early
            warm = cpool.tile([1, HALF], F32)
            nc.scalar.activation(
                warm[:], selc4_sb[0:1, 0, :HALF].bitcast(F32), AF.Sqrt
            )

            psum_ctx = (
                tc.tile_pool(name="pss", bufs=2, space="PSUM"),
                tc.tile_pool(name="plog", bufs=3, space="PSUM"),
                tc.tile_pool(name="pf", bufs=2, space="PSUM"),
            )
            pss = psum_ctx[0].__enter__()
            plog = psum_ctx[1].__enter__()
            pf = psum_ctx[2].__enter__()

            sss = []
            fx_sbs = []
            lgs_sbs = {}

            def emit_fb_out(kk):
                half = kk // HALF
                ii = kk % HALF
                fb = pf.tile([C, BLOC], F32, tag="fb", name=f"fb{kk}")
                nc.tensor.matmul(
                    fb[:],
                    lhsT=selc4_sb[:, ii, :],
                    rhs=fx_sbs[half][:],
                    start=True, stop=True,
                    skip_group_check=True,
                )
                o_sb = opool.tile([C, BLOC], BF16, tag="o", name=f"o{kk}")
                nc.vector.tensor_tensor(o_sb[:], lgs_sbs[kk][:], fb[:], MULT)
                eng = nc.sync if kk < 6 else nc.scalar
                eng.dma_start(out[kk], o_sb[:])

            def emit_fchain(half):
                ss = sss[half]
                recx = fpool.tile([HALF, BLOC], F32, tag="recx",
                                  name=f"recx{half}")
                scr = fpool.tile([HALF, BLOC], F32, tag="rscr",
                                 name=f"rscr{half}")
                nc.vector.reciprocal_approx_accurate(recx[:], ss[:], scr[:])
                fx = fpool.tile([HALF, BLOC], F32R, tag="fx", name=f"fx{half}")
                nc.scalar.activation(fx[:], recx[:], AF.Sqrt)
                fx_sbs.append(fx)

            def ss_mm(ss, x2, i, jp):
                nc.tensor.matmul(
                    ss[:],
                    lhsT=sel4_sb[:, :, i, :],
                    rhs=x2[:, jp],
                    start=(i == 0 and jp == 0),
                    stop=(i == HALF - 1 and jp == JP - 1),
                    perf_mode=DROW,
                    skip_group_check=True,
                )

            for k in range(K):
                half, i = divmod(k, HALF)
                if i == 0:
                    ss = pss.tile([HALF, BLOC], F32, tag="ss", name=f"ss{half}")
                    sss.append(ss)
                ss = sss[half]
                if k == HALF:
                    # h0 f-chain traced here: recip gates on k3's last ss
                    emit_fchain(0)
                # squares into fp8 row-pairs: ACT jp0, DVE jp1 (both from
                # the first-arriving x half), DVE jp2, gpsimd jp3 (DVE for
                # k7: gpsimd is slower and jp3(k7) gates the tail chain)
                x2 = x2pool.tile([P, JP, 2, BLOC], F8, tag="x2", name=f"x2_{k}")
                nc.scalar.activation(x2[:, 0:1], x_sbs[k][:, 0:2], AF.Square)
                nc.vector.tensor_tensor(
                    x2[:, 1:2], x_sbs[k][:, 2:4], x_sbs[k][:, 2:4], MULT
                )
                nc.vector.tensor_tensor(
                    x2[:, 2:3], x_sbs[k][:, 4:6], x_sbs[k][:, 4:6], MULT
                )
                if k < K - 1:
                    nc.gpsimd.tensor_tensor(
                        x2[:, 3:4], x_sbs[k][:, 6:8], x_sbs[k][:, 6:8], MULT
                    )
                else:
                    nc.vector.tensor_tensor(
                        x2[:, 3:4], x_sbs[k][:, 6:8], x_sbs[k][:, 6:8], MULT
                    )
                # first-half ss + main matmuls can run before the second
                # x half lands
                ss_mm(ss, x2, i, 0)
                ss_mm(ss, x2, i, 1)
                lg = plog.tile([C, BLOC], F32, tag="lg", name=f"lg{k}")
                for j in range(JA):
                    nc.tensor.matmul(
                        lg[:],
                        lhsT=w_ap(k)[:, j, :],
                        rhs=x_sbs[k][:, j],
                        start=(j == 0),
                        stop=False,
                        skip_group_check=True,
                    )
                ss_mm(ss, x2, i, 2)
                ss_mm(ss, x2, i, 3)
                if k == K - 1:
                    # h1 f-chain gates only on k7's ss, traced before the
                    # remaining main matmuls so it overlaps them
                    emit_fchain(1)
                for j in range(JA, DC):
                    nc.tensor.matmul(
                        lg[:],
                        lhsT=w_ap(k)[:, j, :],
                        rhs=x_sbs[k][:, j],
                        start=False,
                        stop=(j == DC - 1),
                        skip_group_check=True,
                    )
                lgs = lgspool.tile([C, BLOC], F32, tag="lgs", name=f"lgs{k}")
                nc.scalar.activation(lgs[:], lg[:], AF.Copy)
                lgs_sbs[k] = lgs
                if half == 1:
                    # deferred half0 outputs: one per k=4..7
                    emit_fb_out(k - HALF)

            for kk in range(HALF, K):
                emit_fb_out(kk)

            for c in reversed(psum_ctx):
                c.__exit__(None, None, None)

    nc.compile()
    return nc


def _host_prep(pooled, active_expert_indices, weights, log_scales):
    idx = np.asarray(active_expert_indices).astype(np.int64)
    pooled = np.asarray(pooled, dtype=np.float32)
    weights = np.asarray(weights, dtype=np.float32)
    log_scales = np.asarray(log_scales, dtype=np.float32)

    # x: [B,K,D] -> bf16 -> per-core [K, P, DC, BLOC]  (k, d, j, b)
    pb = pooled.astype(NPBF16)
    xt_all = np.ascontiguousarray(
        pb.reshape(N_CORES, BLOC, K, DC, P).transpose(0, 2, 4, 3, 1)
    )
    # w_eff: gather + fold cosine normalizer and clamped logit scale
    wg = weights[idx]  # [K, C, D]
    nrm = np.sqrt(np.sum(wg * wg, axis=-1, keepdims=True))
    scale = np.minimum(np.exp(log_scales[idx]), 100.0)[:, None, None]
    weff = (wg / np.maximum(nrm, 1e-12) * scale).astype(NPBF16)
    # [K,C,D] -> [K/2, P, 2, DC, C] k-pair packs
    wt = np.ascontiguousarray(
        weff.reshape(K // 2, 2, C, DC, P).transpose(0, 4, 1, 3, 2)
    )

    sel4 = np.zeros((P, 2, HALF, HALF), NPF8)
    for i in range(HALF):
        sel4[:, :, i, i] = 1.0
    selc4 = np.zeros((HALF, HALF, C), np.float32)
    for i in range(HALF):
        selc4[i, i, :] = 1.0

    shared = {"wt": wt, "sel4": sel4, "selc4": selc4}
    return [dict(shared, xt=np.ascontiguousarray(xt_all[co]))
            for co in range(N_CORES)]


def kernel(pooled, active_expert_indices, weights, log_scales):
    global LAST_RESULT, LAST_WALL_NS
    if "nc" not in _CACHE:
        _CACHE["nc"] = _build()
    nc = _CACHE["nc"]

    in_maps = _host_prep(pooled, active_expert_indices, weights, log_scales)

    t0 = time.perf_counter_ns()
    res = run_bass_kernel_spmd(nc, in_maps, core_ids=list(range(N_CORES)))
    LAST_WALL_NS = time.perf_counter_ns() - t0
    LAST_RESULT = res

    full = np.stack(
        [res.results[co]["out"].astype(np.float32) for co in range(N_CORES)]
    )
    return np.ascontiguousarray(
        full.transpose(0, 3, 1, 2).reshape(B, K, C)
    )
